# revision 1
# baseline (speedup 1.0000x reference)
"""Trainium2 Bass kernel for nn_AutoregressiveLAMDecoder (B=16384, D=1024, H=8, NT=4, NC=16).

Strategy (data-parallel over 8 cores, R=2048 rows/core), with exact algebraic
restructuring validated against the reference:
  - cross-attention collapses (softmax over a single key): ca = mem @ (Wo_ca@Wv_ca).T + b
  - ff_w2 and out_w fold into a (16, 2048) matrix
  - ln1/ln3 affine params fold into the adjacent weight matrices
  - self-attention q/k/v depend only on discrete token ids (17 per position):
    scores become a precomputed (4096, 128) table gathered per row by combo id;
    softmax is a tiny row-major pass; the attention apply is ONE K=128 matmul
    per (position, head) using zero-padded packed value tables and a
    softmax-weighted one-hot rhs.
All on-device matmuls in bf16 with fp32 PSUM accumulation; layernorm statistics,
softmax, and residual adds in fp32.
"""
import sys
for _p in ('/opt/trn_rl_repo', '/root/.axon_site/_ro/trn_rl_repo'):
    if _p not in sys.path:
        sys.path.insert(0, _p)

import math
import os as _os
import numpy as np
import ml_dtypes

B, D, H = 16384, 1024, 8
NT, NC = 4, 16
DFF = 2048
DH = D // H
N_CORES = 8
R = B // N_CORES          # rows per core
BF16 = ml_dtypes.bfloat16

_CACHE = {}


# ---------------------------------------------------------------- host math
def _ln_rows(x, g, b, eps=1e-5):
    m = x.mean(-1, keepdims=True)
    v = ((x - m) ** 2).mean(-1, keepdims=True)
    return (x - m) / np.sqrt(v + eps) * g + b


def _host_precompute(i):
    """Weight-only folds and tables (float64 internally)."""
    f = {k: np.asarray(v, np.float64) for k, v in i.items()
         if np.asarray(v).dtype != np.int64 and np.asarray(v).dtype != np.int32}
    P = {}
    P['WcpT'] = (f['cp_w'] * f['cp_ln_g'][None, :]).T            # (din, dout)
    b_cp = f['cp_b'] + f['cp_w'] @ f['cp_ln_b']
    P['b_cp'] = b_cp
    W_ca = f['ca_wo'] @ f['ca_wv']
    P['WcaT'] = W_ca.T
    P['b_ca'] = f['ca_wo'] @ f['ca_bv'] + f['ca_bo']
    P['W2pT'] = (f['out_w'] @ f['ff_w2']).T                       # (2048, 16)
    P['b_out2'] = f['out_b'] + f['out_w'] @ f['ff_b2']
    P['W1T'] = (f['ff_w1'] * f['ln3_g'][None, :]).T               # (1024, 2048)
    P['b1'] = f['ff_b1'] + f['ff_w1'] @ f['ln3_b']
    P['WoT'] = f['sa_wo'].T
    P['bo'] = f['sa_bo']
    P['OwT'] = f['out_w'].T                                       # (1024, 16)

    E = np.stack([f['tok_emb'] + f['pos_emb'][p][None, :] for p in range(NT)])
    P['E'] = E                                                    # (4, 17, D)
    L = np.stack([_ln_rows(E[p], f['ln1_g'], f['ln1_b']) for p in range(NT)])
    Q = L @ f['sa_wq'].T + f['sa_bq']
    K = L @ f['sa_wk'].T + f['sa_bk']
    V = L @ f['sa_wv'].T + f['sa_bv']
    P['V'] = V

    # padded full-combo score table, col = p*32 + h*4 + j, per-(p,h) max-0 norm
    S_full = np.full((4096, 128), -30.0)
    t0, t1, t2 = np.meshgrid(np.arange(16), np.arange(16), np.arange(16),
                             indexing='ij')
    ctok = np.stack([np.full(4096, 16), t0.ravel(), t1.ravel(), t2.ravel()], 1)
    for p in range(NT):
        for h in range(H):
            cols = []
            for j in range(p + 1):
                qh = Q[p][:, h*DH:(h+1)*DH]
                kh = K[j][:, h*DH:(h+1)*DH]
                S = (qh @ kh.T) / math.sqrt(DH)
                S_full[:, p*32 + h*4 + j] = S[ctok[:, p], ctok[:, j]]
                cols.append(p*32 + h*4 + j)
            gmax = S_full[:, cols].max(1, keepdims=True)
            for c in cols:
                S_full[:, c] -= gmax[:, 0]
    P['S_full'] = S_full

    # packed value tables per head: rows 32j..32j+17 = V[j][:, head], zero pad
    Vpk = np.zeros((H, 128, DH))
    for h in range(H):
        for j in range(NT):
            Vpk[h, 32*j:32*j+17, :] = V[j][:, h*DH:(h+1)*DH]
    P['Vpk'] = Vpk

    # packed position-embedding tables: rows 32p..32p+17 = E[p], zero pad
    Epk = np.zeros((NT, 128, D))
    for p in range(NT):
        Epk[p, 32*p:32*p+17, :] = E[p]
    P['Epk'] = Epk

    # broadcast selectors: SEL[p*8+h][r, m] = 1 iff r == p*32 + h*4 + m//32
    SEL = np.zeros((32, 128, 128))
    m = np.arange(128)
    for p in range(NT):
        for h in range(H):
            SEL[p*8 + h, :, :] = (np.arange(128)[:, None]
                                  == (p*32 + h*4 + m//32)[None, :])
    P['SEL'] = SEL
    return P


def _per_core_inputs(P, ctx_shard, tg_shard):
    """Batch-dependent marshalling for one core."""
    r = ctx_shard.shape[0]
    tok = np.concatenate([np.full((r, 1), 16, np.int64), tg_shard[:, :3]], 1)
    cidx = (tg_shard[:, 0] * 256 + tg_shard[:, 1] * 16
            + tg_shard[:, 2]).astype(np.int16)
    # wrapped gather indices: [128, r//16], row p slot s = cidx[s*16 + p%16]
    # wrapped per 1024-row gather block (dma_gather crashes above 1024 idxs)
    gb = min(r, 1024)
    sidx = np.zeros((128, r // 16), np.int16)
    for g in range(r // gb):
        blk = cidx[g*gb:(g+1)*gb]
        for p in range(128):
            sidx[p, g*(gb//16):(g+1)*(gb//16)] = blk[
                np.arange(gb // 16) * 16 + (p % 16)]
    # packed one-hot [128, r] bf16: rows 32j..32j+17 block j one-hot of tok_j
    oh = np.zeros((128, r), np.float32)
    rows = np.arange(r)
    for j in range(NT):
        oh[32*j + tok[:, j], rows] = 1.0
    return {
        'ctx': np.ascontiguousarray(ctx_shard, np.float32),
        'oh': oh.astype(BF16),
        'sidx': sidx,
    }


def _shared_inputs(P):
    bf = lambda a: np.ascontiguousarray(a, BF16)
    f32 = lambda a: np.ascontiguousarray(a, np.float32)
    col = lambda b, n: f32(np.asarray(b).reshape(n, 128).T)   # [128, n]
    return {
        'sfull': f32(P['S_full']),
        'wcp': bf(P['WcpT']), 'wca': bf(P['WcaT']), 'wo': bf(P['WoT']),
        'w1': bf(P['W1T']), 'w2p': bf(P['W2pT']), 'oww': bf(P['OwT']),
        'epk': bf(P['Epk']), 'vpk': bf(P['Vpk']), 'sel': bf(P['SEL']),
        'bcp_s2': col(P['b_cp'] / math.sqrt(2.0), 8),
        'bcp': col(P['b_cp'], 8),
        'bca': col(P['b_ca'], 8),
        'bbo': col(P['bo'], 8),
        'bb1': col(P['b1'], 16),
        'bout': f32(np.asarray(P['b_out2']).reshape(16, 1)),
    }


# ---------------------------------------------------------------- device build
def build_nc(rows=R, rep=1, use_gather=True, stage_limit=99):
    import concourse.bass as bass
    import concourse.mybir as mybir
    from concourse import bacc
    from concourse.tile import TileContext
    from concourse.masks import make_identity

    dt = mybir.dt
    AF = mybir.ActivationFunctionType
    OP = mybir.AluOpType
    AP = bass.AP

    NCH = rows // 512
    NTL = rows // 128

    nc = bacc.Bacc("TRN2", target_bir_lowering=False, debug=False,
                   num_devices=N_CORES)
    din = lambda n, s, d: nc.dram_tensor(n, s, d, kind="ExternalInput").ap()
    ctx = din("ctx", [rows, D], dt.float32)
    oh_d = din("oh", [128, rows], dt.bfloat16)
    sidx_d = din("sidx", [128, rows // 16], dt.int16)
    sfull = din("sfull", [4096, 128], dt.float32)
    wcp_d = din("wcp", [D, D], dt.bfloat16)
    wca_d = din("wca", [D, D], dt.bfloat16)
    wo_d = din("wo", [D, D], dt.bfloat16)
    w1_d = din("w1", [D, DFF], dt.bfloat16)
    w2p_d = din("w2p", [DFF, 16], dt.bfloat16)
    ow_d = din("oww", [D, 16], dt.bfloat16)
    epk_d = din("epk", [NT, 128, D], dt.bfloat16)
    vpk_d = din("vpk", [H, 128, DH], dt.bfloat16)
    sel_d = din("sel", [32, 128, 128], dt.bfloat16)
    bcp2_d = din("bcp_s2", [128, 8], dt.float32)
    bcp_d = din("bcp", [128, 8], dt.float32)
    bca_d = din("bca", [128, 8], dt.float32)
    bbo_d = din("bbo", [128, 8], dt.float32)
    bb1_d = din("bb1", [128, 16], dt.float32)
    bout_d = din("bout", [16, 1], dt.float32)
    out_d = nc.dram_tensor("out", [rows, NT, 16], dt.float32,
                           kind="ExternalOutput").ap()

    def brc(ap, n):
        """Append a broadcast (step-0) free dim of size n to an AP."""
        return AP(ap.tensor, ap.offset, ap.ap + [[0, n]])

    with TileContext(nc) as tc:
        with (
            tc.tile_pool(name="wp", bufs=1) as wp,
            tc.tile_pool(name="res", bufs=1) as res,
            tc.tile_pool(name="pmm", bufs=4, space="PSUM") as pmm,
            tc.tile_pool(name="pst", bufs=2, space="PSUM") as pst,
            tc.tile_pool(name="pO", bufs=1, space="PSUM") as pO,
            tc.tile_pool(name="ptp", bufs=1, space="PSUM") as ptp,
        ):
            # ---- constants / weights
            ident_b = wp.tile([128, 128], dt.bfloat16, tag="identb")
            make_identity(nc, ident_b)
            ident_f = wp.tile([128, 128], dt.float32, tag="identf")
            make_identity(nc, ident_f)
            ones_k = wp.tile([128, 1], dt.bfloat16, tag="onesk")
            nc.vector.memset(ones_k, 1.0)
            ones_m = wp.tile([1, 128], dt.bfloat16, tag="onesm")
            nc.vector.memset(ones_m, 1.0)
            eps128 = wp.tile([128, 1], dt.float32, tag="eps128")
            nc.vector.memset(eps128, 1e-5)
            eps1 = wp.tile([1, 1], dt.float32, tag="eps1")
            nc.vector.memset(eps1, 1e-5)

            wcp = wp.tile([128, 8, D], dt.bfloat16, tag="wcp")
            nc.sync.dma_start(wcp[:], wcp_d.rearrange("(k p) n -> p k n", p=128))
            wca = wp.tile([128, 8, D], dt.bfloat16, tag="wca")
            nc.sync.dma_start(wca[:], wca_d.rearrange("(k p) n -> p k n", p=128))
            wo = wp.tile([128, 8, D], dt.bfloat16, tag="wo")
            nc.sync.dma_start(wo[:], wo_d.rearrange("(k p) n -> p k n", p=128))
            w1 = wp.tile([128, 8, DFF], dt.bfloat16, tag="w1")
            nc.sync.dma_start(w1[:], w1_d.rearrange("(k p) n -> p k n", p=128))
            w2p = wp.tile([128, 16, 16], dt.bfloat16, tag="w2p")
            nc.sync.dma_start(w2p[:], w2p_d.rearrange("(k p) n -> p k n", p=128))
            oww = wp.tile([128, 8, 16], dt.bfloat16, tag="oww")
            nc.sync.dma_start(oww[:], ow_d.rearrange("(k p) n -> p k n", p=128))
            epk = wp.tile([128, NT, D], dt.bfloat16, tag="epk")
            nc.sync.dma_start(epk[:], epk_d.rearrange("q p n -> p q n"))
            vpk = wp.tile([128, H, DH], dt.bfloat16, tag="vpk")
            nc.sync.dma_start(vpk[:], vpk_d.rearrange("h p n -> p h n"))
            sel = wp.tile([128, 32, 128], dt.bfloat16, tag="sel")
            nc.sync.dma_start(sel[:], sel_d.rearrange("s p n -> p s n"))
            oh = wp.tile([128, rows], dt.bfloat16, tag="oh")
            nc.sync.dma_start(oh[:], oh_d[:])
            bcp2 = wp.tile([128, 8], dt.float32, tag="bcp2")
            nc.sync.dma_start(bcp2[:], bcp2_d[:])
            bcp = wp.tile([128, 8], dt.float32, tag="bcp")
            nc.sync.dma_start(bcp[:], bcp_d[:])
            bca = wp.tile([128, 8], dt.float32, tag="bca")
            nc.sync.dma_start(bca[:], bca_d[:])
            bbo = wp.tile([128, 8], dt.float32, tag="bbo")
            nc.sync.dma_start(bbo[:], bbo_d[:])
            bb1 = wp.tile([128, 16], dt.float32, tag="bb1")
            nc.sync.dma_start(bb1[:], bb1_d[:])
            bout = wp.tile([16, 1], dt.float32, tag="bout")
            nc.sync.dma_start(bout[:], bout_d[:])
            sidx = wp.tile([128, rows // 16], dt.int16, tag="sidx")
            nc.sync.dma_start(sidx[:], sidx_d[:])

            aT = res.tile([128, NTL, 128], dt.bfloat16, tag="aT")

            from contextlib import nullcontext
            _loopctx = tc.For_i(0, rep, 1) if rep > 1 else nullcontext()
            with _loopctx:
             if True:
                # ---- score gather + softmax + transpose -> aT (scoped pool)
                with tc.tile_pool(name="smx", bufs=1) as smx:
                    sg = smx.tile([128, NTL, 32, 4], dt.float32, tag="sg")
                    if use_gather:
                        gb = min(rows, 1024)
                        for g in range(rows // gb):
                            nc.gpsimd.dma_gather(
                                out_ap=sg[:, g*(gb//128):(g+1)*(gb//128), :, :]
                                    .rearrange("p t g j -> p t (g j)"),
                                in_ap=sfull,
                                idxs_ap=sidx[:, g*(gb//16):(g+1)*(gb//16)],
                                num_idxs=gb,
                                num_idxs_reg=gb,
                                elem_size=128,
                            )
                    else:
                        nc.vector.memset(sg[:], 0.0)
                    eg = smx.tile([128, NTL, 32, 4], dt.float32, tag="eg")
                    nc.scalar.activation(eg[:], sg[:], AF.Exp)
                    gs = smx.tile([128, NTL, 32], dt.float32, tag="gs")
                    nc.vector.tensor_reduce(gs[:], eg[:],
                                            axis=mybir.AxisListType.X, op=OP.add)
                    gr = smx.tile([128, NTL, 32], dt.float32, tag="gr")
                    nc.vector.reciprocal(gr[:], gs[:])
                    ab = smx.tile([128, NTL, 32, 4], dt.bfloat16, tag="ab")
                    nc.vector.tensor_tensor(ab[:], eg[:], brc(gr[:, :, :], 4),
                                            OP.mult)
                    if stage_limit >= 1:
                        for t in range(NTL):
                            tp = ptp.tile([128, 128], dt.bfloat16, tag="tp")
                            nc.tensor.transpose(tp[:], ab[:, t, :, :], ident_b[:])
                            nc.scalar.copy(aT[:, t, :], tp[:])
                    else:
                        nc.vector.memset(aT[:], 0.0)

                with (
                    tc.tile_pool(name="fm", bufs=1) as fm,
                    tc.tile_pool(name="rl", bufs=2) as rl,
                    tc.tile_pool(name="st", bufs=1) as st,
                ):
                    for chn in range(NCH):
                        c0 = chn * 512
                        # ---- context LN + transpose (4 row-tiles)
                        lnxT = fm.tile([128, 8, 512], dt.bfloat16, tag="lnxT")
                        if stage_limit < 2 or _os.environ.get('KNOTRANS'):
                            nc.vector.memset(lnxT[:], 0.0)
                        for tt in (range(4) if stage_limit >= 2 else []):
                            t = chn * 4 + tt
                            xt = rl.tile([128, D], dt.float32, tag="xt", bufs=2)
                            nc.sync.dma_start(xt[:], ctx[t*128:(t+1)*128, :])
                            _kln = _os.environ.get('KLN', 'full')
                            if _kln == 'triv':
                                xn = rl.tile([128, D], dt.bfloat16, tag="xn",
                                             bufs=1)
                                nc.scalar.copy(xn[:], xt[:])
                                continue
                            s1 = rl.tile([128, 1], dt.float32, tag="s1")
                            nc.vector.tensor_reduce(s1[:], xt[:],
                                                    axis=mybir.AxisListType.X,
                                                    op=OP.add)
                            mu = rl.tile([128, 1], dt.float32, tag="mu")
                            nc.scalar.activation(mu[:], s1[:], AF.Copy, bias=0.0,
                                                 scale=1.0 / D)
                            sqj = rl.tile([128, D], dt.bfloat16, tag="sqj",
                                          bufs=1)
                            ssq = rl.tile([128, 1], dt.float32, tag="ssq")
                            if True:  # tensor_tensor_reduce crashes HW; use 2-op form
                                nc.vector.tensor_tensor(sqj[:], xt[:], xt[:],
                                                        OP.mult)
                                nc.vector.tensor_reduce(
                                    ssq[:], sqj[:], axis=mybir.AxisListType.X,
                                    op=OP.add)
                            else:
                                nc.vector.tensor_tensor_reduce(
                                    out=sqj[:], in0=xt[:], in1=xt[:], scale=1.0,
                                    scalar=0.0, op0=OP.mult, op1=OP.add,
                                    accum_out=ssq[:])
                            if _kln == 'stats1':
                                xn = rl.tile([128, D], dt.bfloat16, tag="xn",
                                             bufs=1)
                                nc.scalar.copy(xn[:], xt[:])
                                continue
                            mu2 = rl.tile([128, 1], dt.float32, tag="mu2")
                            nc.vector.tensor_tensor(mu2[:], mu[:], mu[:], OP.mult)
                            var = rl.tile([128, 1], dt.float32, tag="var")
                            nc.vector.scalar_tensor_tensor(
                                out=var[:], in0=ssq[:], scalar=1.0 / D,
                                in1=mu2[:], op0=OP.mult, op1=OP.subtract)
                            sd = rl.tile([128, 1], dt.float32, tag="sd")
                            nc.scalar.activation(sd[:], var[:], AF.Sqrt,
                                                 bias=eps128[:])
                            rstd = rl.tile([128, 1], dt.float32, tag="rstd")
                            nc.vector.reciprocal(rstd[:], sd[:])
                            mr = rl.tile([128, 1], dt.float32, tag="mr")
                            nc.vector.tensor_tensor(mr[:], mu[:], rstd[:],
                                                    OP.mult)
                            nmr = rl.tile([128, 1], dt.float32, tag="nmr")
                            nc.vector.tensor_scalar(nmr[:], mr[:], -1.0, None,
                                                    OP.mult)
                            xn = rl.tile([128, D], dt.bfloat16, tag="xn", bufs=1)
                            if _kln == 'stats2':
                                nc.scalar.copy(xn[:], xt[:])
                            else:
                                nc.scalar.activation(xn[:], xt[:], AF.Identity,
                                                     bias=nmr[:], scale=rstd[:])
                            for kb in range(8):
                                if _os.environ.get('KNOTRANS'):
                                    continue
                                tp = ptp.tile([128, 128], dt.bfloat16, tag="tp")
                                nc.tensor.transpose(tp[:],
                                                    xn[:, kb*128:(kb+1)*128],
                                                    ident_b[:])
                                nc.scalar.copy(lnxT[:, kb, tt*128:(tt+1)*128],
                                               tp[:])
                        # ---- mem = gelu(cp(lnx))  (exact erf form)
                        mem = fm.tile([128, 8, 512], dt.bfloat16, tag="mem")
                        if stage_limit < 2:
                            nc.vector.memset(mem[:], 0.0)
                        _nomem = bool(_os.environ.get('KNOMEM'))
                        if _nomem:
                            nc.vector.memset(mem[:], 0.0)
                        for mb in (range(8) if stage_limit >= 2 and not _nomem
                                   else []):
                            z = pmm.tile([128, 512], dt.float32, tag="mm")
                            for kb in range(8):
                                nc.tensor.matmul(z[:],
                                                 wcp[:, kb, mb*128:(mb+1)*128],
                                                 lnxT[:, kb, :],
                                                 start=(kb == 0), stop=(kb == 7))
                            e = rl.tile([128, 512], dt.bfloat16, tag="erf",
                                        bufs=2)
                            _erf_f = (AF.Relu if _os.environ.get('KNOERF')
                                      else AF.Erf)
                            nc.scalar.activation(e[:], z[:], _erf_f,
                                                 bias=bcp2[:, mb:mb+1],
                                                 scale=1.0 / math.sqrt(2.0))
                            tz = rl.tile([128, 512], dt.float32, tag="tz",
                                         bufs=2)
                            nc.vector.tensor_scalar(tz[:], z[:], bcp[:, mb:mb+1],
                                                    0.5, OP.add, OP.mult)
                            nc.vector.scalar_tensor_tensor(
                                out=mem[:, mb, :], in0=e[:], scalar=1.0,
                                in1=tz[:], op0=OP.add, op1=OP.mult)
                        # ---- ca = Wca @ mem + bca
                        casb = fm.tile([128, 8, 512], dt.bfloat16, tag="ca")
                        if stage_limit < 3:
                            nc.vector.memset(casb[:], 0.0)
                        for mb in (range(8) if stage_limit >= 3 else []):
                            z = pmm.tile([128, 512], dt.float32, tag="mm")
                            for kb in range(8):
                                nc.tensor.matmul(z[:],
                                                 wca[:, kb, mb*128:(mb+1)*128],
                                                 mem[:, kb, :],
                                                 start=(kb == 0), stop=(kb == 7))
                            nc.scalar.activation(casb[:, mb, :], z[:],
                                                 AF.Identity,
                                                 bias=bca[:, mb:mb+1])

                        stage = fm.tile([128, 4, NT, 16], dt.float32, tag="stage")
                        if stage_limit < 7:
                            nc.vector.memset(stage[:], 0.0)
                        for p in (range(NT) if stage_limit >= 4 else []):
                            # ---- attention apply
                            osb = fm.tile([128, 8, 512], dt.bfloat16, tag="osb")
                            for h in range(H):
                                bc = pmm.tile([128, 512], dt.float32, tag="mm")
                                nc.tensor.matmul(bc[:], sel[:, p*8 + h, :],
                                                 aT[:, chn*4:(chn+1)*4, :],
                                                 start=True, stop=True)
                                wbf = rl.tile([128, 512], dt.bfloat16, tag="wbf",
                                              bufs=2)
                                nc.vector.tensor_tensor(wbf[:], oh[:, c0:c0+512],
                                                        bc[:], OP.mult)
                                ops = pmm.tile([128, 512], dt.float32, tag="mm")
                                nc.tensor.matmul(ops[:], vpk[:, h, :], wbf[:],
                                                 start=True, stop=True)
                                nc.scalar.copy(osb[:, h, :], ops[:])
                            # ---- x2 = Wo@o + Epk@oh + bo + ca
                            x2 = fm.tile([128, 8, 512], dt.bfloat16, tag="x2")
                            if stage_limit < 5:
                                nc.vector.memset(x2[:], 0.0)
                            for mb in (range(8) if stage_limit >= 5 else []):
                                zp = pmm.tile([128, 512], dt.float32, tag="mm")
                                for kb in range(8):
                                    nc.tensor.matmul(
                                        zp[:], wo[:, kb, mb*128:(mb+1)*128],
                                        osb[:, kb, :],
                                        start=(kb == 0), stop=False)
                                nc.tensor.matmul(zp[:],
                                                 epk[:, p, mb*128:(mb+1)*128],
                                                 oh[:, c0:c0+512],
                                                 start=False, stop=True)
                                nc.vector.scalar_tensor_tensor(
                                    out=x2[:, mb, :], in0=zp[:],
                                    scalar=bbo[:, mb:mb+1], in1=casb[:, mb, :],
                                    op0=OP.add, op1=OP.add)
                            if stage_limit < 6:
                                continue
                            # ---- ln3 stats
                            sps = pst.tile([1, 512], dt.float32, tag="stat")
                            qps = pst.tile([1, 512], dt.float32, tag="stat")
                            for kb in range(8):
                                nc.tensor.matmul(sps[:], ones_k[:], x2[:, kb, :],
                                                 start=(kb == 0), stop=(kb == 7))
                                sq = rl.tile([128, 512], dt.bfloat16, tag="sq",
                                             bufs=2)
                                nc.vector.tensor_tensor(sq[:], x2[:, kb, :],
                                                        x2[:, kb, :], OP.mult)
                                nc.tensor.matmul(qps[:], ones_k[:], sq[:],
                                                 start=(kb == 0), stop=(kb == 7))
                            mean = st.tile([1, 512], dt.float32, tag="statf",
                                           bufs=3)
                            nc.scalar.activation(mean[:], sps[:], AF.Copy,
                                                 bias=0.0, scale=1.0 / D)
                            m2 = st.tile([1, 512], dt.float32, tag="statf",
                                         bufs=3)
                            nc.vector.tensor_tensor(m2[:], mean[:], mean[:],
                                                    OP.mult)
                            var3 = st.tile([1, 512], dt.float32, tag="statf",
                                           bufs=3)
                            nc.vector.scalar_tensor_tensor(
                                out=var3[:], in0=qps[:], scalar=1.0 / D,
                                in1=m2[:], op0=OP.mult, op1=OP.subtract)
                            sd3 = st.tile([1, 512], dt.float32, tag="statf",
                                          bufs=3)
                            nc.scalar.activation(sd3[:], var3[:], AF.Sqrt,
                                                 bias=eps1[:])
                            rs3 = st.tile([1, 512], dt.float32, tag="statf",
                                          bufs=3)
                            nc.vector.reciprocal(rs3[:], sd3[:])
                            mbf = st.tile([1, 512], dt.bfloat16, tag="statb",
                                          bufs=2)
                            nc.vector.tensor_copy(mbf[:], mean[:])
                            rbf = st.tile([1, 512], dt.bfloat16, tag="statb",
                                          bufs=2)
                            nc.vector.tensor_copy(rbf[:], rs3[:])
                            mbc = pmm.tile([128, 512], dt.float32, tag="mm")
                            nc.tensor.matmul(mbc[:], ones_m[:], mbf[:],
                                             start=True, stop=True)
                            rbc = pmm.tile([128, 512], dt.float32, tag="mm")
                            nc.tensor.matmul(rbc[:], ones_m[:], rbf[:],
                                             start=True, stop=True)
                            x2n = fm.tile([128, 8, 512], dt.bfloat16, tag="x2n")
                            for kb in range(8):
                                t3 = rl.tile([128, 512], dt.bfloat16, tag="t3",
                                             bufs=2)
                                nc.vector.tensor_tensor(t3[:], x2[:, kb, :],
                                                        mbc[:], OP.subtract)
                                nc.vector.tensor_tensor(x2n[:, kb, :], t3[:],
                                                        rbc[:], OP.mult)
                            if stage_limit < 7:
                                continue
                            # ---- ff1 + relu + folded ff2/out, plus x2 @ OwT
                            Ops = pO.tile([16, 512], dt.float32, tag="O")
                            for kb in range(8):
                                nc.tensor.matmul(Ops[:], oww[:, kb, :],
                                                 x2[:, kb, :],
                                                 start=(kb == 0), stop=False)
                            for fb in range(16):
                                hps = pmm.tile([128, 512], dt.float32, tag="mm")
                                for kb in range(8):
                                    nc.tensor.matmul(
                                        hps[:], w1[:, kb, fb*128:(fb+1)*128],
                                        x2n[:, kb, :],
                                        start=(kb == 0), stop=(kb == 7))
                                hsb = rl.tile([128, 512], dt.bfloat16, tag="hsb",
                                              bufs=2)
                                nc.scalar.activation(hsb[:], hps[:], AF.Relu,
                                                     bias=bb1[:, fb:fb+1])
                                nc.tensor.matmul(Ops[:], w2p[:, fb, :], hsb[:],
                                                 start=False, stop=(fb == 15))
                            Osb = rl.tile([16, 512], dt.float32, tag="Osb",
                                          bufs=1)
                            nc.scalar.activation(Osb[:], Ops[:], AF.Identity,
                                                 bias=bout[:, 0:1])
                            for s4 in range(4):
                                tpo = ptp.tile([128, 16], dt.float32, tag="tp")
                                nc.tensor.transpose(tpo[:],
                                                    Osb[:, s4*128:(s4+1)*128],
                                                    ident_f[:16, :16])
                                nc.scalar.copy(stage[:, s4, p, :], tpo[:])
                        for s4 in range(4):
                            g0 = c0 + s4 * 128
                            nc.sync.dma_start(out_d[g0:g0+128, :, :],
                                              stage[:, s4, :, :])

    nc.compile()
    return nc


# ---------------------------------------------------------------- PJRT runner
class _SpmdRunner:
    def __init__(self, nc, n_cores):
        import jax
        import numpy as _np
        from jax.sharding import Mesh, PartitionSpec
        from jax.experimental.shard_map import shard_map
        import concourse.mybir as mybir
        from concourse import bass2jax
        bass2jax.install_neuronx_cc_hook()
        self.jax = jax
        self.n_cores = n_cores
        partition_name = (nc.partition_id_tensor.name
                          if nc.partition_id_tensor else None)
        in_names, out_names, out_avals, zero_outs = [], [], [], []
        for alloc in nc.m.functions[0].allocations:
            if not isinstance(alloc, mybir.MemoryLocationSet):
                continue
            name = alloc.memorylocations[0].name
            if alloc.kind == "ExternalInput":
                if name != partition_name:
                    in_names.append(name)
            elif alloc.kind == "ExternalOutput":
                shape = tuple(alloc.tensor_shape)
                dtype = mybir.dt.np(alloc.dtype)
                out_names.append(name)
                out_avals.append(jax.core.ShapedArray(shape, dtype))
                zero_outs.append(_np.zeros(shape, dtype))
        self.in_names, self.out_names = in_names, out_names
        self.out_avals, self.zero_outs = out_avals, zero_outs
        n_params, n_outs = len(in_names), len(out_avals)
        all_in = in_names + out_names
        if partition_name is not None:
            all_in.append(partition_name)

        def _body(*args):
            operands = list(args)
            if partition_name is not None:
                operands.append(bass2jax.partition_id_tensor())
            return tuple(bass2jax._bass_exec_p.bind(
                *operands, out_avals=tuple(out_avals),
                in_names=tuple(all_in), out_names=tuple(out_names),
                lowering_input_output_aliases=(),
                sim_require_finite=True, sim_require_nnan=True, nc=nc))

        devices = jax.devices()[:n_cores]
        mesh = Mesh(_np.asarray(devices), ("core",))
        self.sharded = jax.jit(
            shard_map(_body, mesh=mesh,
                      in_specs=(PartitionSpec("core"),) * (n_params + n_outs),
                      out_specs=(PartitionSpec("core"),) * n_outs,
                      check_rep=False),
            donate_argnums=tuple(range(n_params, n_params + n_outs)),
            keep_unused=True)

    def concat_inputs(self, in_maps):
        import numpy as _np
        per_core = [[_np.asarray(m[n]) for n in self.in_names] for m in in_maps]
        return [_np.concatenate([per_core[c][i] for c in range(self.n_cores)], 0)
                for i in range(len(self.in_names))]

    def zeros(self):
        import numpy as _np
        return [_np.zeros((self.n_cores * z.shape[0], *z.shape[1:]), z.dtype)
                for z in self.zero_outs]

    def run_concat(self, concat_in):
        out_arrs = self.sharded(*concat_in, *self.zeros())
        import numpy as _np
        return [_np.asarray(a) for a in out_arrs]


def _get_runner(rows=R, rep=1):
    key = (rows, rep)
    if key not in _CACHE:
        nc = build_nc(rows, rep)
        _CACHE[key] = _SpmdRunner(nc, N_CORES)
    return _CACHE[key]


# ---------------------------------------------------------------- public entry
def kernel(**inputs):
    ctx_full = np.asarray(inputs['context'], np.float32)
    tg_full = np.asarray(inputs['targets']).astype(np.int64)
    assert ctx_full.shape == (B, D)
    P = _host_precompute(inputs)
    shared = _shared_inputs(P)
    runner = _get_runner(R, 1)
    in_maps = []
    for c in range(N_CORES):
        m = dict(shared)
        m.update(_per_core_inputs(P, ctx_full[c*R:(c+1)*R], tg_full[c*R:(c+1)*R]))
        in_maps.append(m)
    concat_in = runner.concat_inputs(in_maps)
    outs = runner.run_concat(concat_in)
    logits = outs[0].reshape(N_CORES * R, NT, 16).astype(np.float32)
    return logits



# revision 4
# speedup vs baseline: 78.2757x; 78.2757x over previous
"""Trainium2 Bass kernel for nn_AutoregressiveLAMDecoder (B=16384, D=1024, H=8, NT=4, NC=16).

Data-parallel over 8 cores (R=2048 rows/core). Exact algebraic restructure:
  - cross-attention collapses (softmax over one key): ca = mem @ (Wo_ca@Wv_ca).T + b
  - the whole self-attention block output + token/pos embedding depends only on
    the discrete token prefix (<= 4096 combos per position): precomputed on host
    into per-position BASE tables, gathered per row on device (transposed layout)
  - ff_w2/out_w fold into a (2048, 16) matrix; ln1/ln3 affines fold into weights
  - layernorms computed in the transposed (feature-partition) layout via
    ones-vector matmul reductions; no 128x128 transposes anywhere
All matmuls bf16 with fp32 PSUM accumulation; stats and softmax f32 on host.
"""
import sys
for _p in ('/opt/trn_rl_repo', '/root/.axon_site/_ro/trn_rl_repo'):
    if _p not in sys.path:
        sys.path.insert(0, _p)

import math
import numpy as np
import ml_dtypes

B, D, H = 16384, 1024, 8
NT, NC = 4, 16
DFF = 2048
DH = D // H
N_CORES = 8
R = B // N_CORES          # rows per core
BF16 = ml_dtypes.bfloat16

_CACHE = {}


# ---------------------------------------------------------------- host math
def _ln_rows(x, g, b, eps=1e-5):
    m = x.mean(-1, keepdims=True)
    v = ((x - m) ** 2).mean(-1, keepdims=True)
    return (x - m) / np.sqrt(v + eps) * g + b


def _host_precompute(i):
    """Weight-only folds and per-combo BASE tables (float64/float32)."""
    f = {k: np.asarray(v, np.float64) for k, v in i.items()
         if np.asarray(v).dtype not in (np.int64, np.int32)}
    P = {}
    P['WcpT'] = (f['cp_w'] * f['cp_ln_g'][None, :]).T            # (din, dout)
    P['b_cp'] = f['cp_b'] + f['cp_w'] @ f['cp_ln_b']
    W_ca = f['ca_wo'] @ f['ca_wv']
    P['WcaT'] = W_ca.T
    P['b_ca'] = f['ca_wo'] @ f['ca_bv'] + f['ca_bo']
    P['W2pT'] = (f['out_w'] @ f['ff_w2']).T                       # (2048, 16)
    P['b_out2'] = f['out_b'] + f['out_w'] @ f['ff_b2']
    P['W1T'] = (f['ff_w1'] * f['ln3_g'][None, :]).T               # (1024, 2048)
    P['b1'] = f['ff_b1'] + f['ff_w1'] @ f['ln3_b']
    P['OwT'] = f['out_w'].T                                       # (1024, 16)

    E = np.stack([f['tok_emb'] + f['pos_emb'][p][None, :] for p in range(NT)])
    L = np.stack([_ln_rows(E[p], f['ln1_g'], f['ln1_b']) for p in range(NT)])
    Q = (L @ f['sa_wq'].T + f['sa_bq']).reshape(NT, NC + 1, H, DH)
    K = (L @ f['sa_wk'].T + f['sa_bk']).reshape(NT, NC + 1, H, DH)
    V = (L @ f['sa_wv'].T + f['sa_bv']).reshape(NT, NC + 1, H, DH)

    # BASE_p[combo] = E[p][qtok] + SA_out(combo) @ wo.T + bo, where combo
    # encodes targets t_0..t_{p-1} (position p attends to shifted[0..p]).
    bases = []
    for p in range(NT):
        S = NC ** p
        digits = np.arange(S)
        ctoks = np.empty((S, p + 1), np.int64)
        ctoks[:, 0] = NC                                      # start token
        for j in range(1, p + 1):
            ctoks[:, j] = (digits // (NC ** (p - j))) % NC    # t_{j-1}
        qtok = ctoks[:, p]
        # scores s[n, h, j] = Q[p][qtok]·K[j][ctoks_j] / sqrt(dh)
        s = np.empty((S, H, p + 1))
        for j in range(p + 1):
            s[:, :, j] = np.einsum('nhd,nhd->nh', Q[p][qtok],
                                   K[j][ctoks[:, j]]) / math.sqrt(DH)
        s -= s.max(-1, keepdims=True)
        a = np.exp(s)
        a /= a.sum(-1, keepdims=True)
        o = np.zeros((S, H, DH))
        for j in range(p + 1):
            o += a[:, :, j:j+1] * V[j][ctoks[:, j]]
        sa = o.reshape(S, D) @ f['sa_wo'].T + f['sa_bo']
        bases.append((E[p][qtok] + sa).astype(np.float32))
    P['base0'] = bases[0][0]                                  # (1024,)
    P['base1'] = bases[1]                                     # (16, 1024)
    P['base2'] = bases[2]                                     # (256, 1024)
    P['base3'] = bases[3]                                     # (4096, 1024)
    return P


def _shared_inputs(P):
    bf = lambda a: np.ascontiguousarray(a, BF16)
    f32 = lambda a: np.ascontiguousarray(a, np.float32)
    col = lambda b, n: f32(np.asarray(b).reshape(n, 128).T)   # [128, n]
    return {
        'wcp': bf(P['WcpT']), 'wca': bf(P['WcaT']),
        'w1': bf(P['W1T']), 'w2p': bf(P['W2pT']), 'oww': bf(P['OwT']),
        'base1': bf(P['base1']), 'base2': bf(P['base2']),
        'base3': bf(P['base3']),
        'bcp': col(P['b_cp'], 8),
        'bca': col(P['b_ca'], 8),
        'b0c': col(P['base0'], 8),
        'bb1': col(P['b1'], 16),
        'bout': f32(np.asarray(P['b_out2']).reshape(16, 1)),
    }


def _per_core_inputs(P, ctx_shard, tg_shard):
    """Batch-dependent marshalling for one core."""
    r = ctx_shard.shape[0]
    t0 = tg_shard[:, 0].astype(np.int64)
    t1 = tg_shard[:, 1].astype(np.int64)
    t2 = tg_shard[:, 2].astype(np.int64)
    idxs = {'gi1': t0, 'gi2': t0 * 16 + t1, 'gi3': t0 * 256 + t1 * 16 + t2}
    out = {'ctxT': np.ascontiguousarray(ctx_shard.T, dtype=BF16)}
    s16 = np.arange(32) * 16
    for k, idx in idxs.items():
        w = np.zeros((128, r // 16), np.int16)
        for c in range(r // 512):
            blk = idx[c*512:(c+1)*512]
            for q in range(128):
                w[q, c*32:(c+1)*32] = blk[s16 + q % 16]
        out[k] = w
    return out


def make_in_maps(inputs):
    ctx_full = np.asarray(inputs['context'], np.float32)
    tg_full = np.asarray(inputs['targets']).astype(np.int64)
    P = _host_precompute(inputs)
    shared = _shared_inputs(P)
    in_maps = []
    for c in range(N_CORES):
        m = dict(shared)
        m.update(_per_core_inputs(P, ctx_full[c*R:(c+1)*R],
                                  tg_full[c*R:(c+1)*R]))
        in_maps.append(m)
    return in_maps


# ---------------------------------------------------------------- device build
def build_nc(rows=R, rep=1):
    import concourse.bass as bass
    import concourse.mybir as mybir
    from concourse import bacc
    from concourse.tile import TileContext

    dt = mybir.dt
    AF = mybir.ActivationFunctionType
    OP = mybir.AluOpType

    NCH = rows // 512

    nc = bacc.Bacc("TRN2", target_bir_lowering=False, debug=False,
                   num_devices=N_CORES)
    din = lambda n, s, d: nc.dram_tensor(n, s, d, kind="ExternalInput").ap()
    ctxT_d = din("ctxT", [D, rows], dt.bfloat16)
    gi1_d = din("gi1", [128, rows // 16], dt.int16)
    gi2_d = din("gi2", [128, rows // 16], dt.int16)
    gi3_d = din("gi3", [128, rows // 16], dt.int16)
    wcp_d = din("wcp", [D, D], dt.bfloat16)
    wca_d = din("wca", [D, D], dt.bfloat16)
    w1_d = din("w1", [D, DFF], dt.bfloat16)
    w2p_d = din("w2p", [DFF, 16], dt.bfloat16)
    ow_d = din("oww", [D, 16], dt.bfloat16)
    base1_d = din("base1", [16, D], dt.bfloat16)
    base2_d = din("base2", [256, D], dt.bfloat16)
    base3_d = din("base3", [4096, D], dt.bfloat16)
    bcp_d = din("bcp", [128, 8], dt.float32)
    bca_d = din("bca", [128, 8], dt.float32)
    b0c_d = din("b0c", [128, 8], dt.float32)
    bb1_d = din("bb1", [128, 16], dt.float32)
    bout_d = din("bout", [16, 1], dt.float32)
    out_d = nc.dram_tensor("out", [rows, NT, 16], dt.float32,
                           kind="ExternalOutput").ap()
    base_d = [None, base1_d, base2_d, base3_d]
    gi_d = [None, gi1_d, gi2_d, gi3_d]

    with TileContext(nc) as tc:
        with (
            tc.tile_pool(name="wp", bufs=1) as wp,
            tc.tile_pool(name="bt", bufs=3) as btp,
            tc.tile_pool(name="fm", bufs=2) as fm,
            tc.tile_pool(name="rl", bufs=2) as rl,
            tc.tile_pool(name="st", bufs=2) as st,
            tc.tile_pool(name="pmm", bufs=4, space="PSUM") as pmm,
            tc.tile_pool(name="pst", bufs=2, space="PSUM") as pst,
            tc.tile_pool(name="pO", bufs=1, space="PSUM") as pO,
            tc.tile_pool(name="ptp", bufs=1, space="PSUM") as ptp,
        ):
            # ---- constants / weights
            from concourse.masks import make_identity
            ident_f = wp.tile([128, 128], dt.float32, tag="identf")
            make_identity(nc, ident_f)
            ones_k = wp.tile([128, 1], dt.bfloat16, tag="onesk")
            nc.vector.memset(ones_k, 1.0)
            ones_m = wp.tile([1, 128], dt.bfloat16, tag="onesm")
            nc.vector.memset(ones_m, 1.0)
            eps1 = wp.tile([1, 1], dt.float32, tag="eps1")
            nc.vector.memset(eps1, 1e-5)

            wcp = wp.tile([128, 8, D], dt.bfloat16, tag="wcp")
            nc.sync.dma_start(wcp[:], wcp_d.rearrange("(k p) n -> p k n", p=128))
            wca = wp.tile([128, 8, D], dt.bfloat16, tag="wca")
            nc.sync.dma_start(wca[:], wca_d.rearrange("(k p) n -> p k n", p=128))
            w1 = wp.tile([128, 8, DFF], dt.bfloat16, tag="w1")
            nc.sync.dma_start(w1[:], w1_d.rearrange("(k p) n -> p k n", p=128))
            w2p = wp.tile([128, 16, 16], dt.bfloat16, tag="w2p")
            nc.sync.dma_start(w2p[:], w2p_d.rearrange("(k p) n -> p k n", p=128))
            oww = wp.tile([128, 8, 16], dt.bfloat16, tag="oww")
            nc.sync.dma_start(oww[:], ow_d.rearrange("(k p) n -> p k n", p=128))
            bcp = wp.tile([128, 8], dt.float32, tag="bcp")
            nc.sync.dma_start(bcp[:], bcp_d[:])
            bca = wp.tile([128, 8], dt.float32, tag="bca")
            nc.sync.dma_start(bca[:], bca_d[:])
            b0c = wp.tile([128, 8], dt.float32, tag="b0c")
            nc.sync.dma_start(b0c[:], b0c_d[:])
            bb1 = wp.tile([128, 16], dt.float32, tag="bb1")
            nc.sync.dma_start(bb1[:], bb1_d[:])
            bout = wp.tile([16, 1], dt.float32, tag="bout")
            nc.sync.dma_start(bout[:], bout_d[:])
            gi = [None] * 4
            for p in (1, 2, 3):
                gtile = wp.tile([128, rows // 16], dt.int16, tag=f"gi{p}")
                nc.sync.dma_start(gtile[:], gi_d[p][:])
                gi[p] = gtile

            def row_stats(src_tiles, sq_tag):
                """src: list of 8 [128,512] bf16 tiles (or [128,8,512] views).
                Returns (mu_b, nmu_b, rs_b) bf16 [128,512] broadcast tiles."""
                sps = pst.tile([1, 512], dt.float32, tag="stat")
                qps = pst.tile([1, 512], dt.float32, tag="stat")
                for kb in range(8):
                    xt = src_tiles(kb)
                    nc.tensor.matmul(sps[:], ones_k[:], xt,
                                     start=(kb == 0), stop=(kb == 7))
                    sq = rl.tile([128, 512], dt.bfloat16, tag=sq_tag, bufs=2)
                    nc.vector.tensor_tensor(sq[:], xt, xt, OP.mult)
                    nc.tensor.matmul(qps[:], ones_k[:], sq[:],
                                     start=(kb == 0), stop=(kb == 7))
                mean = st.tile([1, 512], dt.float32, tag="statf", bufs=3)
                nc.scalar.activation(mean[:], sps[:], AF.Copy, bias=0.0,
                                     scale=1.0 / D)
                m2 = st.tile([1, 512], dt.float32, tag="statf", bufs=3)
                nc.vector.tensor_tensor(m2[:], mean[:], mean[:], OP.mult)
                var = st.tile([1, 512], dt.float32, tag="statf", bufs=3)
                nc.vector.scalar_tensor_tensor(
                    out=var[:], in0=qps[:], scalar=1.0 / D,
                    in1=m2[:], op0=OP.mult, op1=OP.subtract)
                sd = st.tile([1, 512], dt.float32, tag="statf", bufs=3)
                nc.scalar.activation(sd[:], var[:], AF.Sqrt, bias=eps1[:])
                rs = st.tile([1, 512], dt.float32, tag="statf", bufs=3)
                nc.vector.reciprocal(rs[:], sd[:])
                mbf = st.tile([1, 512], dt.bfloat16, tag="statb", bufs=2)
                nc.vector.tensor_copy(mbf[:], mean[:])
                rbf = st.tile([1, 512], dt.bfloat16, tag="statb", bufs=2)
                nc.vector.tensor_copy(rbf[:], rs[:])
                mbc = pmm.tile([128, 512], dt.float32, tag="mm")
                nc.tensor.matmul(mbc[:], ones_m[:], mbf[:], start=True,
                                 stop=True)
                rbc = pmm.tile([128, 512], dt.float32, tag="mm")
                nc.tensor.matmul(rbc[:], ones_m[:], rbf[:], start=True,
                                 stop=True)
                mu_b = st.tile([128, 512], dt.bfloat16, tag="mub", bufs=2)
                nc.scalar.copy(mu_b[:], mbc[:])
                rs_b = st.tile([128, 512], dt.bfloat16, tag="rsb", bufs=2)
                nc.scalar.copy(rs_b[:], rbc[:])
                return mu_b, rs_b

            for chn in range(NCH):
                c0 = chn * 512
                # ---- load ctxT slice, LN via matmul stats
                xt = fm.tile([128, 8, 512], dt.bfloat16, tag="xt")
                for kb in range(8):
                    nc.sync.dma_start(
                        xt[:, kb, :], ctxT_d[kb*128:(kb+1)*128, c0:c0+512])
                mu_b, rs_b = row_stats(lambda kb: xt[:, kb, :], "sqc")
                lnx = fm.tile([128, 8, 512], dt.bfloat16, tag="lnx")
                for kb in range(8):
                    t = rl.tile([128, 512], dt.bfloat16, tag="t", bufs=2)
                    nc.vector.tensor_tensor(t[:], xt[:, kb, :], mu_b[:],
                                            OP.subtract)
                    nc.vector.tensor_tensor(lnx[:, kb, :], t[:], rs_b[:],
                                            OP.mult)
                # ---- mem = gelu(cp(lnx))
                mem = fm.tile([128, 8, 512], dt.bfloat16, tag="mem", bufs=1)
                for mb in range(8):
                    z = pmm.tile([128, 512], dt.float32, tag="mm")
                    for kb in range(8):
                        nc.tensor.matmul(z[:], wcp[:, kb, mb*128:(mb+1)*128],
                                         lnx[:, kb, :],
                                         start=(kb == 0), stop=(kb == 7))
                    nc.scalar.activation(mem[:, mb, :], z[:], AF.Gelu,
                                         bias=bcp[:, mb:mb+1])
                # ---- ca = Wca @ mem + bca
                casb = fm.tile([128, 8, 512], dt.bfloat16, tag="ca")
                for mb in range(8):
                    z = pmm.tile([128, 512], dt.float32, tag="mm")
                    for kb in range(8):
                        nc.tensor.matmul(z[:], wca[:, kb, mb*128:(mb+1)*128],
                                         mem[:, kb, :],
                                         start=(kb == 0), stop=(kb == 7))
                    nc.scalar.activation(casb[:, mb, :], z[:], AF.Identity,
                                         bias=bca[:, mb:mb+1])

                stage = fm.tile([128, 4, NT, 16], dt.float32, tag="stage")
                for p in range(NT):
                    # ---- x2 = base_p(combo) + ca
                    x2 = fm.tile([128, 8, 512], dt.bfloat16, tag="x2", bufs=1)
                    if p == 0:
                        for kb in range(8):
                            nc.vector.tensor_scalar(
                                x2[:, kb, :], casb[:, kb, :],
                                b0c[:, kb:kb+1], None, OP.add)
                    else:
                        bt = btp.tile([128, 8, 512], dt.bfloat16, tag="bt")
                        nc.gpsimd.dma_gather(
                            out_ap=bt[:],
                            in_ap=base_d[p],
                            idxs_ap=gi[p][:, chn*32:(chn+1)*32],
                            num_idxs=512,
                            num_idxs_reg=512,
                            elem_size=D,
                            transpose=True,
                        )
                        for kb in range(8):
                            nc.vector.tensor_tensor(
                                x2[:, kb, :], bt[:, kb, :], casb[:, kb, :],
                                OP.add)
                    # ---- ln3 stats + normalize
                    mu3, rs3 = row_stats(lambda kb: x2[:, kb, :], "sq2")
                    x2n = fm.tile([128, 8, 512], dt.bfloat16, tag="x2n", bufs=1)
                    for kb in range(8):
                        t = rl.tile([128, 512], dt.bfloat16, tag="t", bufs=2)
                        nc.vector.tensor_tensor(t[:], x2[:, kb, :], mu3[:],
                                                OP.subtract)
                        nc.vector.tensor_tensor(x2n[:, kb, :], t[:], rs3[:],
                                                OP.mult)
                    # ---- out = x2 @ Ow + relu(x2n @ W1 + b1) @ W2p + bout
                    Ops = pO.tile([16, 512], dt.float32, tag="O")
                    for kb in range(8):
                        nc.tensor.matmul(Ops[:], oww[:, kb, :], x2[:, kb, :],
                                         start=(kb == 0), stop=False)
                    for fb in range(16):
                        hps = pmm.tile([128, 512], dt.float32, tag="mm")
                        for kb in range(8):
                            nc.tensor.matmul(
                                hps[:], w1[:, kb, fb*128:(fb+1)*128],
                                x2n[:, kb, :],
                                start=(kb == 0), stop=(kb == 7))
                        hsb = rl.tile([128, 512], dt.bfloat16, tag="hsb",
                                      bufs=2)
                        nc.scalar.activation(hsb[:], hps[:], AF.Relu,
                                             bias=bb1[:, fb:fb+1])
                        nc.tensor.matmul(Ops[:], w2p[:, fb, :], hsb[:],
                                         start=False, stop=(fb == 15))
                    Osb = rl.tile([16, 512], dt.float32, tag="Osb", bufs=2)
                    nc.scalar.activation(Osb[:], Ops[:], AF.Identity,
                                         bias=bout[:, 0:1])
                    for s4 in range(4):
                        tpo = ptp.tile([128, 16], dt.float32, tag="tp")
                        nc.tensor.transpose(tpo[:], Osb[:, s4*128:(s4+1)*128],
                                            ident_f[:16, :16])
                        nc.scalar.copy(stage[:, s4, p, :], tpo[:])
                for s4 in range(4):
                    g0 = c0 + s4 * 128
                    nc.sync.dma_start(out_d[g0:g0+128, :, :],
                                      stage[:, s4, :, :])

    nc.compile()
    return nc


# ---------------------------------------------------------------- PJRT runner
class _SpmdRunner:
    def __init__(self, nc, n_cores):
        import jax
        import numpy as _np
        from jax.sharding import Mesh, PartitionSpec
        from jax.experimental.shard_map import shard_map
        import concourse.mybir as mybir
        from concourse import bass2jax
        bass2jax.install_neuronx_cc_hook()
        self.jax = jax
        self.n_cores = n_cores
        partition_name = (nc.partition_id_tensor.name
                          if nc.partition_id_tensor else None)
        in_names, out_names, out_avals, zero_outs = [], [], [], []
        for alloc in nc.m.functions[0].allocations:
            if not isinstance(alloc, mybir.MemoryLocationSet):
                continue
            name = alloc.memorylocations[0].name
            if alloc.kind == "ExternalInput":
                if name != partition_name:
                    in_names.append(name)
            elif alloc.kind == "ExternalOutput":
                shape = tuple(alloc.tensor_shape)
                dtype = mybir.dt.np(alloc.dtype)
                out_names.append(name)
                out_avals.append(jax.core.ShapedArray(shape, dtype))
                zero_outs.append(_np.zeros(shape, dtype))
        self.in_names, self.out_names = in_names, out_names
        self.out_avals, self.zero_outs = out_avals, zero_outs
        n_params, n_outs = len(in_names), len(out_avals)
        all_in = in_names + out_names
        if partition_name is not None:
            all_in.append(partition_name)

        def _body(*args):
            operands = list(args)
            if partition_name is not None:
                operands.append(bass2jax.partition_id_tensor())
            return tuple(bass2jax._bass_exec_p.bind(
                *operands, out_avals=tuple(out_avals),
                in_names=tuple(all_in), out_names=tuple(out_names),
                lowering_input_output_aliases=(),
                sim_require_finite=True, sim_require_nnan=True, nc=nc))

        devices = jax.devices()[:n_cores]
        mesh = Mesh(_np.asarray(devices), ("core",))
        self.sharded = jax.jit(
            shard_map(_body, mesh=mesh,
                      in_specs=(PartitionSpec("core"),) * (n_params + n_outs),
                      out_specs=(PartitionSpec("core"),) * n_outs,
                      check_rep=False),
            donate_argnums=tuple(range(n_params, n_params + n_outs)),
            keep_unused=True)

    def concat_inputs(self, in_maps):
        import numpy as _np
        per_core = [[_np.asarray(m[n]) for n in self.in_names] for m in in_maps]
        return [_np.concatenate([per_core[c][i] for c in range(self.n_cores)], 0)
                for i in range(len(self.in_names))]

    def zeros(self):
        import numpy as _np
        return [_np.zeros((self.n_cores * z.shape[0], *z.shape[1:]), z.dtype)
                for z in self.zero_outs]

    def run_concat(self, concat_in):
        out_arrs = self.sharded(*concat_in, *self.zeros())
        import numpy as _np
        return [_np.asarray(a) for a in out_arrs]


def _get_runner(rows=R, rep=1):
    key = (rows, rep)
    if key not in _CACHE:
        nc = build_nc(rows, rep)
        _CACHE[key] = _SpmdRunner(nc, N_CORES)
    return _CACHE[key]


# ---------------------------------------------------------------- public entry
def kernel(**inputs):
    ctx_full = np.asarray(inputs['context'], np.float32)
    assert ctx_full.shape == (B, D)
    runner = _get_runner(R, 1)
    in_maps = make_in_maps(inputs)
    concat_in = runner.concat_inputs(in_maps)
    outs = runner.run_concat(concat_in)
    logits = outs[0].reshape(B, NT, 16).astype(np.float32)
    return logits
